# revision 39
# baseline (speedup 1.0000x reference)
"""Multi-head attention (B=2, S=2048, D=1024, H=16, causal, interleaved RoPE)
on 8 Trainium2 NeuronCores.

Sharding: tensor-parallel over heads - 2 heads (128 channels) per core.
Each core computes its Q/K/V projections, RoPE, causal attention, and a
row-parallel partial of the output projection; the host sums the bf16
partials in fp32.

All matmuls in bf16 with fp32 PSUM accumulation. Key structure:
  * x^T is pre-transposed and cast to bf16 on the host (block-major layout)
    so no on-device transposes are needed for the projections.
  * Q/K projection weights are host-permuted so each head's dims are
    [evens(32), odds(32)]; the RoPE pair-swap is then a 32-partition-block
    permutation done with ONE PE matmul against a 0/1 permutation matrix.
  * Attention uses the S^T layout: scores psum [k(128part), q(512)] via
    matmul(lhsT=K^T, rhs=Q^T), the two heads concurrent on disjoint PE row
    groups writing the two banks of one [128,1024] psum tile; ONE exp over
    both heads; causal masking multiplies only the [128,2x128] diagonal
    strip by a triangular constant (DVE). PV via matmul(lhsT=V_aug, rhs=P^T)
    with V_aug = [ones | 63 zero-pad | v dims] per head: the softmax
    denominator lands on psum partition 0 and y-rows on partitions 64-127
    (legal DVE base). 1/denom via fast-approx reciprocal, broadcast over
    partitions with a K=1 PE matmul.
  * Fine-grained software pipelining: projection blocks, softmax epilogues
    and output-projection chunks are emitted as small filler bundles between
    the ks-steps of the attention loop, so the PE queue always holds
    independent work while ACT chews exp (keeps HAM at 2.4 GHz).
"""

from collections import deque

import numpy as np
import ml_dtypes

import concourse.bacc as bacc
import concourse.mybir as mybir
import concourse.tile as tile
from concourse.bass_utils import run_bass_kernel_spmd
from concourse.masks import make_identity

P = 128
B, S, D = 2, 2048, 1024
H, DH = 16, 64
NROWS = B * S            # 4096 flattened rows
CH = 128                 # channels per core (2 heads)
RB = 512                 # row block for projections / q tiles
NRB = NROWS // RB        # 8
DSUB = D // P            # 8 contraction subtiles
KSUB = NROWS // P        # 32 k subtiles (128 rows each)
QT_PER_B = S // RB       # 4 q tiles per batch
ROPE_BASE = 10000.0

f32 = mybir.dt.float32
bf16 = mybir.dt.bfloat16
nbf16 = ml_dtypes.bfloat16

_CACHE = {}


def _build():
    nc = bacc.Bacc("TRN2", target_bir_lowering=False)

    xT_ext = nc.declare_dram_parameter("xT", [P, NRB * DSUB * RB], bf16,
                                       isOutput=False)
    wqT_ext = nc.declare_dram_parameter("wqT", [P, DSUB * CH], bf16,
                                        isOutput=False)
    wkT_ext = nc.declare_dram_parameter("wkT", [P, DSUB * CH], bf16,
                                        isOutput=False)
    wvT_ext = nc.declare_dram_parameter("wvT", [P, DSUB * CH], bf16,
                                        isOutput=False)
    woT_ext = nc.declare_dram_parameter("woT", [CH, D], bf16, isOutput=False)
    bq_ext = nc.declare_dram_parameter("bq", [CH, 1], f32, isOutput=False)
    bk_ext = nc.declare_dram_parameter("bk", [CH, 1], f32, isOutput=False)
    bv_ext = nc.declare_dram_parameter("bv", [CH, 1], f32, isOutput=False)
    cc_ext = nc.declare_dram_parameter("cc2", [P, 2 * S], bf16, isOutput=False)
    ss_ext = nc.declare_dram_parameter("ss2", [P, 2 * S], bf16, isOutput=False)
    tri_ext = nc.declare_dram_parameter("tri", [P, 2 * P], bf16,
                                        isOutput=False)
    psw_ext = nc.declare_dram_parameter("pswm", [P, P], bf16, isOutput=False)
    out_ext = nc.declare_dram_parameter("out", [NROWS, D], bf16, isOutput=True)

    with tile.TileContext(nc) as tc:
        with (
            tc.tile_pool(name="const", bufs=1) as cpool,
            tc.tile_pool(name="xpool", bufs=NRB) as xpool,
            tc.tile_pool(name="big", bufs=1) as big,
            tc.tile_pool(name="work", bufs=3) as work,
            tc.tile_pool(name="ptp", bufs=8) as ptp,
            tc.tile_pool(name="small", bufs=2) as small,
            tc.tile_pool(name="obp", bufs=3) as obp,
            tc.tile_pool(name="psum", bufs=2, space="PSUM") as psum,
            tc.tile_pool(name="psacc", bufs=2, space="PSUM") as psacc,
        ):
            # ---- input DMAs, ordered so block 0's deps land first ----
            wq_sb = cpool.tile([P, DSUB, CH], bf16, tag="wq")
            wk_sb = cpool.tile([P, DSUB, CH], bf16, tag="wk")
            wv_sb = cpool.tile([P, DSUB, CH], bf16, tag="wv")
            nc.sync.dma_start(wq_sb[:].rearrange("p d c -> p (d c)"),
                              wqT_ext[:])
            nc.sync.dma_start(wk_sb[:].rearrange("p d c -> p (d c)"),
                              wkT_ext[:])
            xTb = []

            def load_xt(rt, split=False):
                xt = xpool.tile([P, DSUB, RB], bf16, tag="xT", name=f"xT{rt}")
                o = rt * DSUB * RB
                if split:
                    h = DSUB * RB // 2
                    nc.sync.dma_start(
                        xt[:, 0:DSUB // 2].rearrange("p d c -> p (d c)"),
                        xT_ext[:, o:o + h])
                    nc.sync.dma_start(
                        xt[:, DSUB // 2:].rearrange("p d c -> p (d c)"),
                        xT_ext[:, o + h:o + 2 * h])
                else:
                    nc.sync.dma_start(
                        xt[:].rearrange("p d c -> p (d c)"),
                        xT_ext[:, o:o + DSUB * RB])
                xTb.append(xt)

            load_xt(0, split=True)
            psw_sb = cpool.tile([P, P], bf16, tag="pswm")
            nc.sync.dma_start(psw_sb[:], psw_ext[:])
            bq_sb = cpool.tile([CH, 1], f32, tag="bq")
            nc.sync.dma_start(bq_sb[:], bq_ext[:])
            bk_sb = cpool.tile([CH, 1], f32, tag="bk")
            nc.sync.dma_start(bk_sb[:], bk_ext[:])
            bv_sb = cpool.tile([CH, 1], f32, tag="bv")
            nc.sync.dma_start(bv_sb[:], bv_ext[:])
            cc_sb = cpool.tile([P, 2, S], bf16, tag="cc")
            nc.sync.dma_start(cc_sb[:].rearrange("p a c -> p (a c)"), cc_ext[:])
            ss_sb = cpool.tile([P, 2, S], bf16, tag="ss")
            nc.sync.dma_start(ss_sb[:].rearrange("p a c -> p (a c)"), ss_ext[:])
            load_xt(1)
            nc.sync.dma_start(wv_sb[:].rearrange("p d c -> p (d c)"),
                              wvT_ext[:])
            tri_sb = cpool.tile([P, 2 * P], bf16, tag="tri")
            nc.sync.dma_start(tri_sb[:], tri_ext[:])
            for rt in range(2, NRB):
                load_xt(rt)
            wo_sb = cpool.tile([CH, D], bf16, tag="wo")
            nc.sync.dma_start(wo_sb[:, 0:512], woT_ext[:, 0:512])
            nc.sync.dma_start(wo_sb[:, 512:1024], woT_ext[:, 512:1024])

            # ---- constants ----
            ones_f = cpool.tile([P, P], f32, tag="onesf")
            nc.vector.memset(ones_f[:], 1.0)
            ones_b = cpool.tile([P, P], bf16, tag="onesb")
            nc.vector.tensor_copy(ones_b[:], ones_f[:])
            ident_f = cpool.tile([P, P], f32, tag="identf")
            make_identity(nc, ident_f[:])
            ident = cpool.tile([P, P], bf16, tag="ident")
            nc.vector.tensor_copy(ident[:], ident_f[:])

            # ---- persistent activation tiles ----
            qkT = big.tile([P, 2, NROWS], bf16, tag="qkT")  # [:,0]=q [:,1]=k
            yT = big.tile([P, NROWS], bf16, tag="yT")
            # per head: [ones | 63 pad | 64 v-dims] = 128 cols
            v_sb = big.tile([P, KSUB, 256], bf16, tag="v")

            nc.vector.tensor_copy(
                v_sb[:, :, 0:129:128].rearrange("p a b -> p (a b)"),
                ones_b[:, 0:2 * KSUB])
            nc.vector.memset(v_sb[:, :, 1:64], 0.0)
            nc.vector.memset(v_sb[:, :, 129:192], 0.0)

            # ---------- phase A (projections + RoPE) as filler chunks ------
            def a_chunks(rt):
                sl = slice(rt * RB, (rt + 1) * RB)
                pos = slice((rt % QT_PER_B) * RB, (rt % QT_PER_B + 1) * RB)
                xt = xTb[rt]
                st_ = {}

                def a1():
                    pqk = psacc.tile([P, 1024], f32, tag="acc",
                                     name=f"pqk{rt}")
                    st_["pqk"] = pqk
                    for d in range(4):
                        nc.tensor.matmul(pqk[:, 0:512], wq_sb[:, d], xt[:, d],
                                         start=(d == 0), stop=False)

                def a2():
                    pqk = st_["pqk"]
                    for d in range(4, 8):
                        nc.tensor.matmul(pqk[:, 0:512], wq_sb[:, d], xt[:, d],
                                         start=False, stop=(d == 7))
                    praw = work.tile([P, 2, RB], bf16, tag="praw")
                    st_["praw"] = praw
                    nc.vector.tensor_scalar_add(praw[:, 0], pqk[:, 0:512],
                                                bq_sb[:, 0:1])

                def a3():
                    pqk = st_["pqk"]
                    for d in range(4):
                        nc.tensor.matmul(pqk[:, 512:1024], wk_sb[:, d],
                                         xt[:, d], start=(d == 0), stop=False)

                def a4():
                    pqk = st_["pqk"]
                    for d in range(4, 8):
                        nc.tensor.matmul(pqk[:, 512:1024], wk_sb[:, d],
                                         xt[:, d], start=False, stop=(d == 7))
                    nc.vector.tensor_scalar_add(st_["praw"][:, 1],
                                                pqk[:, 512:1024],
                                                bk_sb[:, 0:1])

                def a5():
                    # swap32 via PE permutation, overwriting the pqk banks
                    pqk, praw = st_["pqk"], st_["praw"]
                    prflat = praw[:].rearrange("p a c -> p (a c)")
                    nc.tensor.matmul(pqk[:, 0:512], psw_sb[:],
                                     prflat[:, 0:512], start=True, stop=True)
                    nc.tensor.matmul(pqk[:, 512:1024], psw_sb[:],
                                     prflat[:, 512:1024], start=True,
                                     stop=True)
                    t1 = work.tile([P, 2, RB], bf16, tag="ropet1")
                    st_["t1"] = t1
                    nc.vector.tensor_mul(t1[:], praw[:], cc_sb[:, :, pos])

                def a6():
                    pqk = st_["pqk"]
                    t2 = work.tile([P, 2, RB], bf16, tag="ropet2")
                    nc.vector.tensor_mul(
                        t2[:], pqk[:].rearrange("p (a c) -> p a c", a=2),
                        ss_sb[:, :, pos])
                    nc.vector.tensor_add(qkT[:, :, sl], st_["t1"][:], t2[:])

                def a7():
                    pqk = st_["pqk"]
                    for d in range(4):
                        nc.tensor.matmul(pqk[:, 0:512], wv_sb[:, d], xt[:, d],
                                         start=(d == 0), stop=False)

                def a8():
                    pqk = st_["pqk"]
                    for d in range(4, 8):
                        nc.tensor.matmul(pqk[:, 0:512], wv_sb[:, d], xt[:, d],
                                         start=False, stop=(d == 7))
                    vr = work.tile([P, RB], bf16, tag="vraw")
                    st_["vr"] = vr
                    nc.vector.tensor_scalar_add(vr[:], pqk[:, 0:512],
                                                bv_sb[:, 0:1])

                def a9():
                    vr = st_["vr"]
                    tpv = psum.tile([P, 512], bf16, tag="st", name=f"tpv{rt}")
                    for rc in range(4):
                        nc.tensor.transpose(tpv[:, rc * P:(rc + 1) * P],
                                            vr[:, rc * P:(rc + 1) * P],
                                            ident[:])
                    tpv_v = tpv[:].rearrange("p (k h c) -> p k h c", k=4, h=2)
                    vdst = (v_sb[:, rt * 4:(rt + 1) * 4, :]
                            .rearrange("p k (h c) -> p k h c", h=2))
                    for hh in range(2):
                        nc.vector.tensor_copy(vdst[:, :, hh, 64:128],
                                              tpv_v[:, :, hh, :])

                return [a1, a2, a3, a4, a5, a6, a7, a8, a9]

            # ---------- softmax epilogue as filler chunks ----------
            def epi_chunks(state):
                b, qt, qcols, pvm = state
                st_ = {}

                def e1():
                    dcp = small.tile([1, 1024], f32, tag="dcp")
                    nc.vector.tensor_copy(dcp[0:1, 0:512], pvm[0:1, 0:512])
                    nc.scalar.copy(dcp[0:1, 512:1024], pvm[0:1, 512:1024])
                    dn = small.tile([1, 1024], f32, tag="dn")
                    nc.vector.reciprocal_approx_fast(dn[:], dcp[:])
                    st_["dn"] = dn

                def e2():
                    # broadcast 1/denom across partitions on the idle gpsimd
                    # engine (source must sit at tile partition 0 on HW)
                    rep = small.tile([P, 1024], f32, tag="rep")
                    nc.gpsimd.partition_broadcast(rep[:], st_["dn"][0:1, :])
                    st_["rep"] = rep

                def e3():
                    rep = st_["rep"]
                    ynorm = small.tile([P, 1024], bf16, tag="ynorm")
                    nc.vector.tensor_mul(ynorm[64:128, 0:512],
                                         pvm[64:128, 0:512],
                                         rep[64:128, 0:512])
                    nc.vector.tensor_mul(ynorm[64:128, 512:1024],
                                         pvm[64:128, 512:1024],
                                         rep[64:128, 512:1024])
                    nc.sync.dma_start(yT[0:64, qcols], ynorm[64:128, 0:512])
                    nc.sync.dma_start(yT[64:128, qcols],
                                      ynorm[64:128, 512:1024])

                return [e1, e2, e3]

            # ---------- phase D (output projection) as filler chunks ------
            def d_chunk(rt):
                def d1():
                    op = psacc.tile([P, 1024], f32, tag="acc", name=f"op{rt}")
                    for ec in range(2):
                        nc.tensor.matmul(op[:, ec * 512:(ec + 1) * 512],
                                         yT[:, rt * P:(rt + 1) * P],
                                         wo_sb[:, ec * 512:(ec + 1) * 512],
                                         start=True, stop=True)
                    ob = obp.tile([P, 1024], bf16, tag="ob")
                    if rt % 2 == 0:
                        nc.vector.tensor_copy(ob[:], op[:])
                    else:
                        nc.scalar.copy(ob[:], op[:])
                    nc.sync.dma_start(out_ext[rt * P:(rt + 1) * P, :], ob[:])
                return d1

            # ---------- attention q-tile with fillers ----------
            def phase_c(b, qt, fillers):
                qcols = slice(b * S + qt * RB, b * S + (qt + 1) * RB)
                nks = qt * 4 + 4
                pvm = psacc.tile([P, 1024], f32, tag="acc",
                                 name=f"pvm{b}_{qt}")
                pts = {}

                def j0_of(ks):
                    m = ks - qt * 4
                    return m * P if m >= 1 else 0

                def emit_pv(kk):
                    jj = j0_of(kk)
                    ptk = pts.pop(kk)
                    for h in range(2):
                        nc.tensor.matmul(
                            pvm[:, h * 512 + jj:(h + 1) * 512],
                            v_sb[:, b * (S // P) + kk, h * P:(h + 1) * P],
                            ptk[:, h, jj:],
                            start=(kk == 0), stop=(kk == nks - 1))

                for ks in range(nks):
                    kcols = slice(b * S + ks * P, b * S + (ks + 1) * P)
                    m = ks - qt * 4
                    j0 = j0_of(ks)
                    qv = slice(b * S + qt * RB + j0, b * S + (qt + 1) * RB)
                    st = psum.tile([P, 1024], f32, tag="st",
                                   name=f"st{b}_{qt}_{ks}")
                    stv = st[:].rearrange("p (h c) -> p h c", h=2)
                    pt = ptp.tile([P, 2, RB], bf16, tag="pt")
                    pts[ks] = pt
                    for h in range(2):
                        hsl = slice(h * 64, (h + 1) * 64)
                        nc.tensor.matmul(st[:, h * 512 + j0:(h + 1) * 512],
                                         qkT[hsl, 1, kcols], qkT[hsl, 0, qv],
                                         start=True, stop=True)
                    nc.scalar.activation(pt[:, :, j0:], stv[:, :, j0:],
                                         mybir.ActivationFunctionType.Exp)
                    if m >= 0:
                        triv = tri_sb[:].rearrange("p (a c) -> p a c", a=2)
                        nc.gpsimd.tensor_mul(pt[:, :, j0:j0 + P],
                                             pt[:, :, j0:j0 + P], triv)
                    if fillers:
                        fillers.popleft()()
                    if ks >= 2:
                        emit_pv(ks - 2)
                for kk in (nks - 2, nks - 1):
                    emit_pv(kk)
                return (b, qt, qcols, pvm)

            # ---------- master schedule ----------
            dq = deque()          # deferred output-projection chunks
            for ch in a_chunks(0):
                ch()
            for ch in a_chunks(1):
                ch()
            prev = None
            for rt in range(NRB):
                b, qt = rt // QT_PER_B, rt % QT_PER_B
                fillers = deque()
                if prev is not None:
                    fillers.extend(epi_chunks(prev))
                if rt < NRB - 2:
                    fillers.extend(a_chunks(rt + 2))
                if rt == 4:
                    # b0 output rows ready after epi(0,3) (in this rt's
                    # fillers); b1 rows 16+4q..19+4q after each epi(1,q)
                    dq.extend(d_chunk(rr) for rr in range(16))
                if rt >= 6:
                    q_done = rt - 6          # epi(1,q_done) in fillers now
                    dq.extend(d_chunk(rr)
                              for rr in range(16 + 4 * q_done,
                                              20 + 4 * q_done))
                nks = qt * 4 + 4
                while len(fillers) < nks + 2 and dq:
                    fillers.append(dq.popleft())
                prev = phase_c(b, qt, fillers)
                while fillers:
                    fillers.popleft()()
            for ch in epi_chunks(prev):
                ch()
            while dq:
                dq.popleft()()
            for rr in range(24, KSUB):
                d_chunk(rr)()

    nc.finalize()
    return nc


def _host_inputs():
    t = np.arange(32, dtype=np.float64)
    inv_freq = 1.0 / (ROPE_BASE ** (2.0 * t / DH))
    pos = np.arange(S, dtype=np.float64)
    ang = pos[None, :] * inv_freq[:, None]          # [32, S]
    cos32 = np.cos(ang).astype(np.float32)
    sin32 = np.sin(ang).astype(np.float32)
    cc = np.tile(cos32, (4, 1))                     # [128, S]
    ss = np.concatenate([-sin32, sin32, -sin32, sin32], axis=0)  # [128, S]
    cc2 = np.concatenate([cc, cc], axis=1)          # [128, 2S] (q|k dup)
    ss2 = np.concatenate([ss, ss], axis=1)

    ii = np.arange(P)[:, None]
    uu = np.arange(P)[None, :]
    tri = (uu >= ii).astype(np.float32)             # [128, 128]
    tri2 = np.concatenate([tri, tri], axis=1)       # [128, 256]

    perm64 = np.concatenate([np.arange(0, 64, 2), np.arange(1, 64, 2)])
    return cc2, ss2, tri2, perm64


def _in_maps(x, Wq, bq, Wk, bk, Wv, bv, Wo):
    cc2, ss2, tri2, perm64 = _host_inputs()
    # swap32 permutation matrix: psw[m,:] = praw[src(m),:], src = xor-32
    # within each 64-block -> pswm[k, m] = 1 iff k == src(m)
    pswm = np.zeros((P, P), dtype=np.float32)
    for m_ in range(P):
        k_ = (m_ & ~63) | ((m_ + 32) & 63)
        pswm[k_, m_] = 1.0
    pswm = pswm.astype(nbf16)
    x2 = np.ascontiguousarray(x.reshape(NROWS, D))
    # xT block-major: xT[p, rt, d, c] = x[512*rt + c, 128*d + p]
    xT = np.ascontiguousarray(
        x2.reshape(NRB, RB, DSUB, P).transpose(3, 0, 2, 1)
        .reshape(P, NRB * DSUB * RB)).astype(nbf16)
    perm128 = np.concatenate([perm64, perm64 + 64])
    cc2b = cc2.astype(nbf16)
    ss2b = ss2.astype(nbf16)
    tri2b = tri2.astype(nbf16)
    def warr(wT):
        # [D, CH] -> [P, DSUB*CH]: w[p, d*CH+c] = wT[d*P+p, c]
        return np.ascontiguousarray(
            wT.reshape(DSUB, P, CH).transpose(1, 0, 2)
            .reshape(P, DSUB * CH)).astype(nbf16)

    maps = []
    for c in range(8):
        sl = slice(c * CH, (c + 1) * CH)
        maps.append({
            "xT": xT,
            "wqT": warr((Wq[sl][perm128] * 0.125).T),
            "wkT": warr(Wk[sl][perm128].T),
            "wvT": warr(Wv[sl].T),
            "woT": np.ascontiguousarray(Wo[:, sl].T).astype(nbf16),
            "bq": (bq[sl][perm128] * 0.125).reshape(CH, 1).copy(),
            "bk": bk[sl][perm128].reshape(CH, 1).copy(),
            "bv": bv[sl].reshape(CH, 1).copy(),
            "cc2": cc2b, "ss2": ss2b, "tri": tri2b, "pswm": pswm,
        })
    return maps


def kernel(x, Wq, bq, Wk, bk, Wv, bv, Wo, bo):
    x = np.asarray(x, dtype=np.float32)
    Wq = np.asarray(Wq, dtype=np.float32)
    Wk = np.asarray(Wk, dtype=np.float32)
    Wv = np.asarray(Wv, dtype=np.float32)
    Wo = np.asarray(Wo, dtype=np.float32)
    bq = np.asarray(bq, dtype=np.float32)
    bk = np.asarray(bk, dtype=np.float32)
    bv = np.asarray(bv, dtype=np.float32)
    bo = np.asarray(bo, dtype=np.float32)

    if "nc" not in _CACHE:
        _CACHE["nc"] = _build()
    nc = _CACHE["nc"]

    res = run_bass_kernel_spmd(nc, _in_maps(x, Wq, bq, Wk, bk, Wv, bv, Wo),
                               core_ids=list(range(8)))
    out = np.zeros((NROWS, D), dtype=np.float32)
    for r in res.results:
        out += r["out"].astype(np.float32)
    out += bo[None, :]
    return out.reshape(B, S, D)


# revision 42
# speedup vs baseline: 1.3102x; 1.3102x over previous
"""Multi-head attention (B=2, S=2048, D=1024, H=16, causal, interleaved RoPE)
on 8 Trainium2 NeuronCores.

Sharding: tensor-parallel over heads - 2 heads (128 channels) per core.
Each core computes its Q/K/V projections, RoPE, causal attention, and a
row-parallel partial of the output projection; the host sums the bf16
partials in fp32.

All matmuls in bf16 with fp32 PSUM accumulation. Key structure:
  * x^T is pre-transposed and cast to bf16 on the host (block-major layout)
    so no on-device transposes are needed for the projections.
  * Q/K projection weights are host-permuted so each head's dims are
    [evens(32), odds(32)]; the RoPE pair-swap is then a 32-partition-block
    permutation done with ONE PE matmul against a 0/1 permutation matrix.
  * Attention uses the S^T layout: scores psum [k(128part), q(512)] via
    matmul(lhsT=K^T, rhs=Q^T), the two heads concurrent on disjoint PE row
    groups writing the two banks of one [128,1024] psum tile; ONE exp over
    both heads; causal masking multiplies only the [128,2x128] diagonal
    strip by a triangular constant (DVE). PV via matmul(lhsT=V_aug, rhs=P^T)
    with V_aug = [ones | 63 zero-pad | v dims] per head: the softmax
    denominator lands on psum partition 0 and y-rows on partitions 64-127
    (legal DVE base). 1/denom via fast-approx reciprocal, broadcast over
    partitions with a K=1 PE matmul.
  * Fine-grained software pipelining: projection blocks, softmax epilogues
    and output-projection chunks are emitted as small filler bundles between
    the ks-steps of the attention loop, so the PE queue always holds
    independent work while ACT chews exp (keeps HAM at 2.4 GHz).
"""

from collections import deque

import numpy as np
import ml_dtypes

import concourse.bacc as bacc
import concourse.mybir as mybir
import concourse.tile as tile
from concourse.bass_utils import run_bass_kernel_spmd
from concourse.masks import make_identity

P = 128
B, S, D = 2, 2048, 1024
H, DH = 16, 64
NROWS = B * S            # 4096 flattened rows
CH = 128                 # channels per core (2 heads)
RB = 512                 # row block for projections / q tiles
NRB = NROWS // RB        # 8
DSUB = D // P            # 8 contraction subtiles
KSUB = NROWS // P        # 32 k subtiles (128 rows each)
QT_PER_B = S // RB       # 4 q tiles per batch
ROPE_BASE = 10000.0

f32 = mybir.dt.float32
bf16 = mybir.dt.bfloat16
nbf16 = ml_dtypes.bfloat16

_CACHE = {}


def _build():
    nc = bacc.Bacc("TRN2", target_bir_lowering=False)

    xT_ext = nc.declare_dram_parameter("xT", [P, NRB * DSUB * RB], bf16,
                                       isOutput=False)
    wqT_ext = nc.declare_dram_parameter("wqT", [P, DSUB * CH], bf16,
                                        isOutput=False)
    wkT_ext = nc.declare_dram_parameter("wkT", [P, DSUB * CH], bf16,
                                        isOutput=False)
    wvT_ext = nc.declare_dram_parameter("wvT", [P, DSUB * CH], bf16,
                                        isOutput=False)
    woT_ext = nc.declare_dram_parameter("woT", [CH, D], bf16, isOutput=False)
    bq_ext = nc.declare_dram_parameter("bq", [CH, 1], f32, isOutput=False)
    bk_ext = nc.declare_dram_parameter("bk", [CH, 1], f32, isOutput=False)
    bv_ext = nc.declare_dram_parameter("bv", [CH, 1], f32, isOutput=False)
    cc_ext = nc.declare_dram_parameter("cc2", [P, 2 * S], bf16, isOutput=False)
    ss_ext = nc.declare_dram_parameter("ss2", [P, 2 * S], bf16, isOutput=False)
    tri_ext = nc.declare_dram_parameter("tri", [P, 2 * P], bf16,
                                        isOutput=False)
    psw_ext = nc.declare_dram_parameter("pswm", [P, P], bf16, isOutput=False)
    out_ext = nc.declare_dram_parameter("out", [NROWS, D], bf16, isOutput=True)

    with tile.TileContext(nc) as tc:
        with (
            tc.tile_pool(name="const", bufs=1) as cpool,
            tc.tile_pool(name="xpool", bufs=NRB) as xpool,
            tc.tile_pool(name="big", bufs=1) as big,
            tc.tile_pool(name="work", bufs=3) as work,
            tc.tile_pool(name="ptp", bufs=8) as ptp,
            tc.tile_pool(name="small", bufs=2) as small,
            tc.tile_pool(name="obp", bufs=3) as obp,
            tc.tile_pool(name="psum", bufs=2, space="PSUM") as psum,
            tc.tile_pool(name="psacc", bufs=2, space="PSUM") as psacc,
        ):
            # ---- input DMAs, ordered so block 0's deps land first ----
            wq_sb = cpool.tile([P, DSUB, CH], bf16, tag="wq")
            wk_sb = cpool.tile([P, DSUB, CH], bf16, tag="wk")
            wv_sb = cpool.tile([P, DSUB, CH], bf16, tag="wv")
            nc.sync.dma_start(wq_sb[:].rearrange("p d c -> p (d c)"),
                              wqT_ext[:])
            nc.sync.dma_start(wk_sb[:].rearrange("p d c -> p (d c)"),
                              wkT_ext[:])
            xTb = []

            def load_xt(rt, split=False):
                xt = xpool.tile([P, DSUB, RB], bf16, tag="xT", name=f"xT{rt}")
                o = rt * DSUB * RB
                if split:
                    h = DSUB * RB // 2
                    nc.sync.dma_start(
                        xt[:, 0:DSUB // 2].rearrange("p d c -> p (d c)"),
                        xT_ext[:, o:o + h])
                    nc.sync.dma_start(
                        xt[:, DSUB // 2:].rearrange("p d c -> p (d c)"),
                        xT_ext[:, o + h:o + 2 * h])
                else:
                    nc.sync.dma_start(
                        xt[:].rearrange("p d c -> p (d c)"),
                        xT_ext[:, o:o + DSUB * RB])
                xTb.append(xt)

            load_xt(0, split=True)
            load_xt(1)
            nc.sync.dma_start(wv_sb[:].rearrange("p d c -> p (d c)"),
                              wvT_ext[:])
            psw_sb = cpool.tile([P, P], bf16, tag="pswm")
            nc.sync.dma_start(psw_sb[:], psw_ext[:])
            bq_sb = cpool.tile([CH, 1], f32, tag="bq")
            nc.sync.dma_start(bq_sb[:], bq_ext[:])
            bk_sb = cpool.tile([CH, 1], f32, tag="bk")
            nc.sync.dma_start(bk_sb[:], bk_ext[:])
            bv_sb = cpool.tile([CH, 1], f32, tag="bv")
            nc.sync.dma_start(bv_sb[:], bv_ext[:])
            cc_sb = cpool.tile([P, 2, S], bf16, tag="cc")
            nc.sync.dma_start(cc_sb[:].rearrange("p a c -> p (a c)"), cc_ext[:])
            ss_sb = cpool.tile([P, 2, S], bf16, tag="ss")
            nc.sync.dma_start(ss_sb[:].rearrange("p a c -> p (a c)"), ss_ext[:])
            tri_sb = cpool.tile([P, 2 * P], bf16, tag="tri")
            nc.sync.dma_start(tri_sb[:], tri_ext[:])
            for rt in range(2, NRB):
                load_xt(rt)
            wo_sb = cpool.tile([CH, D], bf16, tag="wo")
            nc.sync.dma_start(wo_sb[:, 0:512], woT_ext[:, 0:512])
            nc.sync.dma_start(wo_sb[:, 512:1024], woT_ext[:, 512:1024])

            # ---- constants ----
            ones_f = cpool.tile([P, P], f32, tag="onesf")
            nc.vector.memset(ones_f[:], 1.0)
            ones_b = cpool.tile([P, P], bf16, tag="onesb")
            nc.vector.tensor_copy(ones_b[:], ones_f[:])
            ident_f = cpool.tile([P, P], f32, tag="identf")
            make_identity(nc, ident_f[:])
            ident = cpool.tile([P, P], bf16, tag="ident")
            nc.vector.tensor_copy(ident[:], ident_f[:])

            # ---- persistent activation tiles ----
            qkT = big.tile([P, 2, NROWS], bf16, tag="qkT")  # [:,0]=q [:,1]=k
            yT = big.tile([P, NROWS], bf16, tag="yT")
            # per head: [ones | 63 pad | 64 v-dims] = 128 cols
            v_sb = big.tile([P, KSUB, 256], bf16, tag="v")

            nc.vector.tensor_copy(
                v_sb[:, :, 0:129:128].rearrange("p a b -> p (a b)"),
                ones_b[:, 0:2 * KSUB])
            nc.vector.memset(v_sb[:, :, 1:64], 0.0)
            nc.vector.memset(v_sb[:, :, 129:192], 0.0)

            # ---------- phase A (projections + RoPE) as filler chunks ------
            def a_chunks(rt):
                sl = slice(rt * RB, (rt + 1) * RB)
                pos = slice((rt % QT_PER_B) * RB, (rt % QT_PER_B + 1) * RB)
                xt = xTb[rt]
                st_ = {}

                def a1():
                    pqk = psacc.tile([P, 1024], f32, tag="acc",
                                     name=f"pqk{rt}")
                    st_["pqk"] = pqk
                    for d in range(4):
                        nc.tensor.matmul(pqk[:, 0:512], wq_sb[:, d], xt[:, d],
                                         start=(d == 0), stop=False)

                def a2():
                    pqk = st_["pqk"]
                    for d in range(4, 8):
                        nc.tensor.matmul(pqk[:, 0:512], wq_sb[:, d], xt[:, d],
                                         start=False, stop=(d == 7))
                    praw = work.tile([P, 2, RB], bf16, tag="praw")
                    st_["praw"] = praw
                    nc.vector.tensor_scalar_add(praw[:, 0], pqk[:, 0:512],
                                                bq_sb[:, 0:1])

                def a3():
                    pqk = st_["pqk"]
                    for d in range(4):
                        nc.tensor.matmul(pqk[:, 512:1024], wk_sb[:, d],
                                         xt[:, d], start=(d == 0), stop=False)

                def a4():
                    pqk = st_["pqk"]
                    for d in range(4, 8):
                        nc.tensor.matmul(pqk[:, 512:1024], wk_sb[:, d],
                                         xt[:, d], start=False, stop=(d == 7))
                    nc.vector.tensor_scalar_add(st_["praw"][:, 1],
                                                pqk[:, 512:1024],
                                                bk_sb[:, 0:1])

                def a5():
                    # swap32 via PE permutation, overwriting the pqk banks
                    pqk, praw = st_["pqk"], st_["praw"]
                    prflat = praw[:].rearrange("p a c -> p (a c)")
                    nc.tensor.matmul(pqk[:, 0:512], psw_sb[:],
                                     prflat[:, 0:512], start=True, stop=True)
                    nc.tensor.matmul(pqk[:, 512:1024], psw_sb[:],
                                     prflat[:, 512:1024], start=True,
                                     stop=True)
                    t1 = work.tile([P, 2, RB], bf16, tag="ropet1")
                    st_["t1"] = t1
                    nc.vector.tensor_mul(t1[:], praw[:], cc_sb[:, :, pos])

                def a6():
                    pqk = st_["pqk"]
                    t2 = work.tile([P, 2, RB], bf16, tag="ropet2")
                    nc.vector.tensor_mul(
                        t2[:], pqk[:].rearrange("p (a c) -> p a c", a=2),
                        ss_sb[:, :, pos])
                    nc.vector.tensor_add(qkT[:, :, sl], st_["t1"][:], t2[:])

                def a7():
                    pqk = st_["pqk"]
                    for d in range(4):
                        nc.tensor.matmul(pqk[:, 0:512], wv_sb[:, d], xt[:, d],
                                         start=(d == 0), stop=False)

                def a8():
                    pqk = st_["pqk"]
                    for d in range(4, 8):
                        nc.tensor.matmul(pqk[:, 0:512], wv_sb[:, d], xt[:, d],
                                         start=False, stop=(d == 7))
                    vr = work.tile([P, RB], bf16, tag="vraw")
                    st_["vr"] = vr
                    nc.vector.tensor_scalar_add(vr[:], pqk[:, 0:512],
                                                bv_sb[:, 0:1])

                def a9():
                    vr = st_["vr"]
                    tpv = psum.tile([P, 512], bf16, tag="st", name=f"tpv{rt}")
                    for rc in range(4):
                        nc.tensor.transpose(tpv[:, rc * P:(rc + 1) * P],
                                            vr[:, rc * P:(rc + 1) * P],
                                            ident[:])
                    tpv_v = tpv[:].rearrange("p (k h c) -> p k h c", k=4, h=2)
                    vdst = (v_sb[:, rt * 4:(rt + 1) * 4, :]
                            .rearrange("p k (h c) -> p k h c", h=2))
                    for hh in range(2):
                        nc.vector.tensor_copy(vdst[:, :, hh, 64:128],
                                              tpv_v[:, :, hh, :])

                return [a1, a2, a3, a4, a5, a6, a7, a8, a9]

            # ---------- softmax epilogue as filler chunks ----------
            def epi_chunks(state):
                b, qt, qcols, pvm = state
                st_ = {}

                def e1():
                    dcp = small.tile([1, 1024], f32, tag="dcp")
                    nc.vector.tensor_copy(dcp[0:1, 0:512], pvm[0:1, 0:512])
                    nc.scalar.copy(dcp[0:1, 512:1024], pvm[0:1, 512:1024])
                    dn = small.tile([1, 1024], f32, tag="dn")
                    nc.vector.reciprocal_approx_fast(dn[:], dcp[:])
                    st_["dn"] = dn

                def e2():
                    # broadcast 1/denom across partitions on the idle gpsimd
                    # engine (source must sit at tile partition 0 on HW)
                    rep = small.tile([P, 1024], f32, tag="rep")
                    nc.gpsimd.partition_broadcast(rep[:], st_["dn"][0:1, :])
                    st_["rep"] = rep

                def e3():
                    rep = st_["rep"]
                    ynorm = small.tile([P, 1024], bf16, tag="ynorm")
                    nc.vector.tensor_mul(ynorm[64:128, 0:512],
                                         pvm[64:128, 0:512],
                                         rep[64:128, 0:512])
                    nc.vector.tensor_mul(ynorm[64:128, 512:1024],
                                         pvm[64:128, 512:1024],
                                         rep[64:128, 512:1024])
                    nc.sync.dma_start(yT[0:64, qcols], ynorm[64:128, 0:512])
                    nc.sync.dma_start(yT[64:128, qcols],
                                      ynorm[64:128, 512:1024])

                return [e1, e2, e3]

            # ---------- phase D (output projection) as filler chunks ------
            def d_chunk(rt):
                def d1():
                    op = psacc.tile([P, 1024], f32, tag="acc", name=f"op{rt}")
                    for ec in range(2):
                        nc.tensor.matmul(op[:, ec * 512:(ec + 1) * 512],
                                         yT[:, rt * P:(rt + 1) * P],
                                         wo_sb[:, ec * 512:(ec + 1) * 512],
                                         start=True, stop=True)
                    ob = obp.tile([P, 1024], bf16, tag="ob")
                    if rt % 2 == 0:
                        nc.vector.tensor_copy(ob[:], op[:])
                    else:
                        nc.scalar.copy(ob[:], op[:])
                    nc.sync.dma_start(out_ext[rt * P:(rt + 1) * P, :], ob[:])
                return d1

            # ---------- attention q-tile with fillers ----------
            def phase_c(b, qt, fillers):
                qcols = slice(b * S + qt * RB, b * S + (qt + 1) * RB)
                nks = qt * 4 + 4
                pvm = psacc.tile([P, 1024], f32, tag="acc",
                                 name=f"pvm{b}_{qt}")
                pts = {}

                def j0_of(ks):
                    m = ks - qt * 4
                    return m * P if m >= 1 else 0

                def emit_pv(kk):
                    jj = j0_of(kk)
                    ptk = pts.pop(kk)
                    for h in range(2):
                        nc.tensor.matmul(
                            pvm[:, h * 512 + jj:(h + 1) * 512],
                            v_sb[:, b * (S // P) + kk, h * P:(h + 1) * P],
                            ptk[:, h, jj:],
                            start=(kk == 0), stop=(kk == nks - 1))

                for ks in range(nks):
                    kcols = slice(b * S + ks * P, b * S + (ks + 1) * P)
                    m = ks - qt * 4
                    j0 = j0_of(ks)
                    qv = slice(b * S + qt * RB + j0, b * S + (qt + 1) * RB)
                    st = psum.tile([P, 1024], f32, tag="st",
                                   name=f"st{b}_{qt}_{ks}")
                    stv = st[:].rearrange("p (h c) -> p h c", h=2)
                    pt = ptp.tile([P, 2, RB], bf16, tag="pt")
                    pts[ks] = pt
                    for h in range(2):
                        hsl = slice(h * 64, (h + 1) * 64)
                        nc.tensor.matmul(st[:, h * 512 + j0:(h + 1) * 512],
                                         qkT[hsl, 1, kcols], qkT[hsl, 0, qv],
                                         start=True, stop=True)
                    nc.scalar.activation(pt[:, :, j0:], stv[:, :, j0:],
                                         mybir.ActivationFunctionType.Exp)
                    if m >= 0:
                        triv = tri_sb[:].rearrange("p (a c) -> p a c", a=2)
                        nc.vector.tensor_mul(pt[:, :, j0:j0 + P],
                                             pt[:, :, j0:j0 + P], triv)
                    if fillers:
                        fillers.popleft()()
                    if ks >= 2:
                        emit_pv(ks - 2)
                for kk in (nks - 2, nks - 1):
                    emit_pv(kk)
                return (b, qt, qcols, pvm)

            # ---------- master schedule ----------
            dq = deque()          # deferred output-projection chunks
            for ch in a_chunks(0):
                ch()
            for ch in a_chunks(1):
                ch()
            prev = None
            for rt in range(NRB):
                b, qt = rt // QT_PER_B, rt % QT_PER_B
                fillers = deque()
                if prev is not None:
                    fillers.extend(epi_chunks(prev))
                if rt < NRB - 2:
                    fillers.extend(a_chunks(rt + 2))
                if rt == 4:
                    # b0 output rows ready after epi(0,3) (in this rt's
                    # fillers); b1 rows 16+4q..19+4q after each epi(1,q)
                    dq.extend(d_chunk(rr) for rr in range(16))
                if rt >= 6:
                    q_done = rt - 6          # epi(1,q_done) in fillers now
                    dq.extend(d_chunk(rr)
                              for rr in range(16 + 4 * q_done,
                                              20 + 4 * q_done))
                nks = qt * 4 + 4
                while len(fillers) < nks + 2 and dq:
                    fillers.append(dq.popleft())
                prev = phase_c(b, qt, fillers)
                while fillers:
                    fillers.popleft()()
            for ch in epi_chunks(prev):
                ch()
            while dq:
                dq.popleft()()
            for rr in range(24, KSUB):
                d_chunk(rr)()

    nc.finalize()
    return nc


def _host_inputs():
    t = np.arange(32, dtype=np.float64)
    inv_freq = 1.0 / (ROPE_BASE ** (2.0 * t / DH))
    pos = np.arange(S, dtype=np.float64)
    ang = pos[None, :] * inv_freq[:, None]          # [32, S]
    cos32 = np.cos(ang).astype(np.float32)
    sin32 = np.sin(ang).astype(np.float32)
    cc = np.tile(cos32, (4, 1))                     # [128, S]
    ss = np.concatenate([-sin32, sin32, -sin32, sin32], axis=0)  # [128, S]
    cc2 = np.concatenate([cc, cc], axis=1)          # [128, 2S] (q|k dup)
    ss2 = np.concatenate([ss, ss], axis=1)

    ii = np.arange(P)[:, None]
    uu = np.arange(P)[None, :]
    tri = (uu >= ii).astype(np.float32)             # [128, 128]
    tri2 = np.concatenate([tri, tri], axis=1)       # [128, 256]

    perm64 = np.concatenate([np.arange(0, 64, 2), np.arange(1, 64, 2)])
    return cc2, ss2, tri2, perm64


def _in_maps(x, Wq, bq, Wk, bk, Wv, bv, Wo):
    cc2, ss2, tri2, perm64 = _host_inputs()
    # swap32 permutation matrix: psw[m,:] = praw[src(m),:], src = xor-32
    # within each 64-block -> pswm[k, m] = 1 iff k == src(m)
    pswm = np.zeros((P, P), dtype=np.float32)
    for m_ in range(P):
        k_ = (m_ & ~63) | ((m_ + 32) & 63)
        pswm[k_, m_] = 1.0
    pswm = pswm.astype(nbf16)
    x2 = np.ascontiguousarray(x.reshape(NROWS, D))
    # xT block-major: xT[p, rt, d, c] = x[512*rt + c, 128*d + p]
    xT = np.ascontiguousarray(
        x2.reshape(NRB, RB, DSUB, P).transpose(3, 0, 2, 1)
        .reshape(P, NRB * DSUB * RB)).astype(nbf16)
    perm128 = np.concatenate([perm64, perm64 + 64])
    cc2b = cc2.astype(nbf16)
    ss2b = ss2.astype(nbf16)
    tri2b = tri2.astype(nbf16)
    def warr(wT):
        # [D, CH] -> [P, DSUB*CH]: w[p, d*CH+c] = wT[d*P+p, c]
        return np.ascontiguousarray(
            wT.reshape(DSUB, P, CH).transpose(1, 0, 2)
            .reshape(P, DSUB * CH)).astype(nbf16)

    maps = []
    for c in range(8):
        sl = slice(c * CH, (c + 1) * CH)
        maps.append({
            "xT": xT,
            "wqT": warr((Wq[sl][perm128] * 0.125).T),
            "wkT": warr(Wk[sl][perm128].T),
            "wvT": warr(Wv[sl].T),
            "woT": np.ascontiguousarray(Wo[:, sl].T).astype(nbf16),
            "bq": (bq[sl][perm128] * 0.125).reshape(CH, 1).copy(),
            "bk": bk[sl][perm128].reshape(CH, 1).copy(),
            "bv": bv[sl].reshape(CH, 1).copy(),
            "cc2": cc2b, "ss2": ss2b, "tri": tri2b, "pswm": pswm,
        })
    return maps


def kernel(x, Wq, bq, Wk, bk, Wv, bv, Wo, bo):
    x = np.asarray(x, dtype=np.float32)
    Wq = np.asarray(Wq, dtype=np.float32)
    Wk = np.asarray(Wk, dtype=np.float32)
    Wv = np.asarray(Wv, dtype=np.float32)
    Wo = np.asarray(Wo, dtype=np.float32)
    bq = np.asarray(bq, dtype=np.float32)
    bk = np.asarray(bk, dtype=np.float32)
    bv = np.asarray(bv, dtype=np.float32)
    bo = np.asarray(bo, dtype=np.float32)

    if "nc" not in _CACHE:
        _CACHE["nc"] = _build()
    nc = _CACHE["nc"]

    res = run_bass_kernel_spmd(nc, _in_maps(x, Wq, bq, Wk, bk, Wv, bv, Wo),
                               core_ids=list(range(8)))
    out = np.zeros((NROWS, D), dtype=np.float32)
    for r in res.results:
        out += r["out"].astype(np.float32)
    out += bo[None, :]
    return out.reshape(B, S, D)
